# revision 16
# baseline (speedup 1.0000x reference)
"""NeuralGCDE Trainium2 kernel, v2.

Data-parallel over batch B=32 across 8 NeuronCores (B_loc=4/core, weights
replicated, no inter-core traffic). Per core the RK4 scan (12 steps x 4
stages) runs on-device.

v2 layout/dtype strategy (vs v1):
  - bf16 everywhere except: PSUM (always fp32), the fp32 master state HZ,
    per-partition biases, and the Eg mask tiles (fp32 to match fp32 PSUM
    operands in the V ops).
  - Fused h|z processing: state HZ [128, 1024] (h cols 0:512, z cols
    512:1024), all RK4 combine ops run [128, 1024] once per step instead
    of twice per state.
  - Full-width elementwise: engine op cost scales with free-dim only, so
    every op uses all 128 partitions.
  - F/G/m/n in i-major rows [128=(i,h)] with chunk-in-columns [1024];
    kh/kz produced chunk-folded [128=(chunk,h), 512] via two offset adds.
  - U outputs pairwise in 2-bank PSUM tiles [128, 1024]; one DVE op per
    pair fuses the PSUM->SBUF move with the Eg mask multiply.
"""
import sys
import os
import numpy as np

if "/opt/trn_rl_repo" not in sys.path:
    sys.path.insert(0, "/opt/trn_rl_repo")

B, N, T, CIN, HID, EMB, KCH = 32, 256, 13, 2, 64, 10, 2
NCORES = 8
BLOC = B // NCORES          # 4
TOK = BLOC * N              # 1024
NSTEP = T - 1               # 12
NSTAGE = 3 * NSTEP + 1      # 37 distinct spline-derivative tensors

_KERNEL_CACHE = {}
LAST_RESULT = None


def _dx_stage_index(t, s):
    if s < 3:
        return 3 * t + s
    return 3 * (t + 1) if (t + 1) < NSTEP else 3 * NSTEP


def _build(n_steps=NSTEP):
    import concourse.bacc as bacc
    import concourse.tile as tile
    from concourse import mybir
    from contextlib import ExitStack

    F32 = mybir.dt.float32
    BF16 = mybir.dt.bfloat16
    AF = mybir.ActivationFunctionType
    ALU = mybir.AluOpType

    nc = bacc.Bacc("TRN2", target_bir_lowering=False, debug=False,
                   num_devices=NCORES)

    def din(name, shape, dt=BF16):
        return nc.dram_tensor(name, shape, dt, kind="ExternalInput").ap()

    WFIN = din("WFIN", [128, 128])
    WFIN3 = din("WFIN3", [128, 128])
    WFINM3 = din("WFINM3", [128, 128])
    WFINM1 = din("WFINM1", [128, 128])
    WFHID = din("WFHID", [128, 128])
    WGIN = din("WGIN", [128, 128])
    WGIN3 = din("WGIN3", [128, 128])
    WGINM3 = din("WGINM3", [128, 128])
    WGINM1 = din("WGINM1", [128, 128])
    WFOUT_A = din("WFOUT_A", [128, 128])
    WFOUT_B = din("WFOUT_B", [128, 128])
    WGOUTD = din("WGOUTD", [128, 128])
    BP2 = din("BP2", [10, 128])
    AT0 = din("AT0", [128, 256])
    AT1 = din("AT1", [128, 256])
    WP = din("WP", [128, 640])
    EGU = din("EGU", [10, 1024])
    EGTP = din("EGTP", [3, 128, 1024], F32)   # DVE-path pair masks
    EGTPB = din("EGTPB", [1, 128, 1024])      # pool-path pair mask (c2=1,j=1)
    EGT4 = din("EGT4", [1, 128, 512], F32)    # c2=0 single mask (DVE)
    EGT4B = din("EGT4B", [1, 128, 512])       # c2=1 single mask (pool)
    IDENT = din("IDENT", [64, 64])
    BFIN2 = din("BFIN2", [128, 1], F32)
    BFHID2 = din("BFHID2", [128, 1], F32)
    BGIN2 = din("BGIN2", [128, 1], F32)
    BFOUT = din("BFOUT", [128, 1], F32)
    BGOUT = din("BGOUT", [128, 1], F32)
    HZF = din("HZF", [128, 1024], F32)
    HZB = din("HZB", [128, 1024])
    DXB = din("DXB", [NSTAGE, 128, 1024])
    ZOUT = nc.dram_tensor("ZOUT", [NSTEP, 128, 512], F32,
                          kind="ExternalOutput").ap()

    _ts = bool(os.environ.get("GCDE_TRACESIM"))
    with tile.TileContext(nc, trace_sim=_ts) as tc, ExitStack() as ctx:
        cp = ctx.enter_context(tc.tile_pool(name="const", bufs=1))
        wk = ctx.enter_context(tc.tile_pool(name="work", bufs=3))
        mk = ctx.enter_context(tc.tile_pool(name="mk", bufs=2))
        st = ctx.enter_context(tc.tile_pool(name="state", bufs=2))
        vp = ctx.enter_context(tc.tile_pool(name="vpool", bufs=4))
        dxp = ctx.enter_context(tc.tile_pool(name="dxp", bufs=3))
        pfp = ctx.enter_context(tc.tile_pool(name="pfp", bufs=2, space="PSUM"))
        plumb = ctx.enter_context(tc.tile_pool(name="plumb", bufs=2,
                                               space="PSUM"))
        pup = ctx.enter_context(tc.tile_pool(name="pup", bufs=2, space="PSUM"))

        def cload(src, shape, tag, dt=BF16):
            t = cp.tile(shape, dt, tag=tag)
            nc.sync.dma_start(t[:], src)
            return t

        wfin = cload(WFIN, [128, 128], "wfin")
        wfin3 = cload(WFIN3, [128, 128], "wfin3")
        wfinm3 = cload(WFINM3, [128, 128], "wfinm3")
        wfinm1 = cload(WFINM1, [128, 128], "wfinm1")
        wfhid = cload(WFHID, [128, 128], "wfhid")
        wgin = cload(WGIN, [128, 128], "wgin")
        wgin3 = cload(WGIN3, [128, 128], "wgin3")
        wginm3 = cload(WGINM3, [128, 128], "wginm3")
        wginm1 = cload(WGINM1, [128, 128], "wginm1")
        wfout_a = cload(WFOUT_A, [128, 128], "wfout_a")
        wfout_b = cload(WFOUT_B, [128, 128], "wfout_b")
        wgoutd = cload(WGOUTD, [128, 128], "wgoutd")
        bp2 = cload(BP2, [10, 128], "bp2")
        at0 = cload(AT0, [128, 256], "at0")
        at1 = cload(AT1, [128, 256], "at1")
        wp = cload(WP, [128, 640], "wp")
        egu = cload(EGU, [10, 1024], "egu")
        ident = cload(IDENT, [64, 64], "ident")
        bfin2 = cload(BFIN2, [128, 1], "bfin2", F32)
        bfhid2 = cload(BFHID2, [128, 1], "bfhid2", F32)
        bgin2 = cload(BGIN2, [128, 1], "bgin2", F32)
        bfout = cload(BFOUT, [128, 1], "bfout", F32)
        bgout = cload(BGOUT, [128, 1], "bgout", F32)
        egtp = []
        for i in range(3):
            t = cp.tile([128, 1024], F32, tag=f"egtp{i}")
            nc.sync.dma_start(t[:], EGTP[i])
            egtp.append(t)
        egtpb = cp.tile([128, 1024], BF16, tag="egtpb")
        nc.sync.dma_start(egtpb[:], EGTPB[0])
        egt4 = cp.tile([128, 512], F32, tag="egt4")
        nc.sync.dma_start(egt4[:], EGT4[0])
        egt4b = cp.tile([128, 512], BF16, tag="egt4b")
        nc.sync.dma_start(egt4b[:], EGT4B[0])

        Hm = st.tile([128, 512], F32, tag="Hm")
        Zm = st.tile([128, 512], F32, tag="Zm")
        hb = st.tile([128, 512], BF16, tag="hb")
        zb = st.tile([128, 512], BF16, tag="zb")
        nc.sync.dma_start(Hm[:], HZF[:, 0:512])
        nc.sync.dma_start(Zm[:], HZF[:, 512:1024])
        nc.sync.dma_start(hb[:], HZB[:, 0:512])
        nc.sync.dma_start(zb[:], HZB[:, 512:1024])

        def _acc_terms(out, terms):
            """Accumulate (weight, rhs, halved) into a PSUM stage input.
            halved k-terms contract per 64-row chunk so each kz/kh half
            unblocks the next stage independently."""
            subs = []
            for w, rhs, halved in terms:
                if halved:
                    subs.append((w[0:64, :], rhs[0:64, :]))
                    subs.append((w[64:128, :], rhs[64:128, :]))
                else:
                    subs.append((w[:], rhs))
            for i, (wap, rap) in enumerate(subs):
                nc.tensor.matmul(out[:], wap, rap, start=(i == 0),
                                 stop=(i == len(subs) - 1),
                                 skip_group_check=True)

        def vf(fterms, gterms, sidx, ktag):
            """fterms/gterms: [(weight_tile, rhs_AP)] accumulated into the
            stage input PSUMs (linearity of the input layers replaces the
            explicit u2/u3/u4 state combines). Returns (kh, kz) [128,512]
            bf16 chunk-folded; kh completes early (f path), kz late."""
            dxb = dxp.tile([128, 1024], BF16, tag="dxb")
            nc.sync.dma_start(dxb[:], DXB[sidx])

            kh = mk.tile([128, 512], BF16, tag="kh" + ktag)
            kz = mk.tile([128, 512], BF16, tag="kz" + ktag)

            # ---- f path ----
            pf1 = pfp.tile([128, 512], F32, tag="pf")
            _acc_terms(pf1, fterms)
            x1 = wk.tile([128, 512], BF16, tag="x1")
            nc.scalar.activation(x1[:], pf1[:], AF.Relu, bias=bfin2[:],
                                 scale=1.0)

            pf2 = pfp.tile([128, 512], F32, tag="pf")
            nc.tensor.matmul(pf2[:], wfhid[:], x1[:], start=True, stop=True)
            x2 = wk.tile([128, 512], BF16, tag="x2")
            nc.scalar.activation(x2[:], pf2[:], AF.Relu, bias=bfhid2[:],
                                 scale=1.0)

            # F in i-major rows, chunk-in-columns [128, 1024]
            Fp = wk.tile([128, 1024], BF16, tag="Fp")
            for half, wo in ((0, wfout_a), (1, wfout_b)):
                pF = pfp.tile([128, 512], F32, tag="pf")
                nc.tensor.matmul(pF[:], wo[:], x2[:], start=True, stop=True)
                nc.scalar.activation(Fp[:, half * 512:(half + 1) * 512],
                                     pF[:], AF.Tanh, bias=bfout[:], scale=1.0)

            # m = F * dX as two equal-base half ops; mh2 = base-0 shadow of
            # the hi half for the i-reduction (walrus: SB+SB operand bases
            # must match).
            m = wk.tile([128, 1024], BF16, tag="m")
            nc.gpsimd.tensor_tensor(m[0:64, :], Fp[0:64, :], dxb[0:64, :],
                                    ALU.mult)
            nc.gpsimd.tensor_tensor(m[64:128, :], Fp[64:128, :],
                                    dxb[64:128, :], ALU.mult)
            mh2 = wk.tile([64, 1024], BF16, tag="mh2")
            nc.gpsimd.tensor_copy(mh2[:], m[64:128, :])
            for c2 in range(2):
                cs = slice(c2 * 512, (c2 + 1) * 512)
                nc.gpsimd.tensor_tensor(kh[c2 * 64:(c2 + 1) * 64, :],
                                        m[0:64, cs], mh2[:, cs], ALU.add)

            # ---- g path ----
            pq = plumb.tile([128, 512], F32, tag="pl")
            _acc_terms(pq, gterms)
            XG = wk.tile([128, 1024], BF16, tag="XG")
            nc.scalar.activation(XG[0:64, 0:512], pq[0:64, :], AF.Relu,
                                 bias=bgin2[0:64], scale=1.0)
            nc.vector.tensor_scalar(XG[0:64, 512:1024], pq[64:128, :],
                                    bgin2[64:128], 0.0, ALU.add, ALU.max)

            # transposes: x [64(i), tok] -> xtt [128(m), (b,mh,i)]
            ptt = plumb.tile([128, 512], BF16, tag="pl")
            for b in range(BLOC):
                for mh in range(2):
                    nc.tensor.transpose(
                        ptt[:, (b * 2 + mh) * 64:(b * 2 + mh + 1) * 64],
                        XG[0:64, b * 256 + mh * 128: b * 256 + (mh + 1) * 128],
                        ident[:],
                    )
            xtt = wk.tile([128, 512], BF16, tag="xtt")
            nc.vector.tensor_copy(xtt[:], ptt[:])

            # support matmul: Ax[i, (b,n)] accumulated over m-halves
            for pi in range(2):
                px = plumb.tile([64, 512], F32, tag="pl")
                for bb in range(2):
                    b = pi * 2 + bb
                    bs = slice(bb * 256, (bb + 1) * 256)
                    nc.tensor.matmul(px[:, bs],
                                     xtt[:, (b * 2) * 64:(b * 2 + 1) * 64],
                                     at0[:], start=True, stop=False)
                    nc.tensor.matmul(px[:, bs],
                                     xtt[:, (b * 2 + 1) * 64:(b * 2 + 2) * 64],
                                     at1[:], start=False, stop=True)
                dst = slice(pi * 512, (pi + 1) * 512)
                if pi == 0:
                    nc.scalar.copy(XG[64:128, dst], px[:])
                else:
                    nc.vector.tensor_copy(XG[64:128, dst], px[:])

            # adaptive conv: U pairs -> masked V -> accumulating projection.
            # c2=0's masks run direct on DVE (PSUM read); c2=1's pair masks
            # go ACT copy-cast -> pool bf16 mult to spread the load.
            Gp = wk.tile([128, 1024], BF16, tag="Gp")
            for c2 in range(2):
                cs = slice(c2 * 512, (c2 + 1) * 512)
                vts = []
                for j in range(2):
                    pu = pup.tile([128, 1024], F32, tag="pu")
                    for k in range(2):
                        c = 2 * j + k
                        nc.tensor.matmul(pu[:, k * 512:(k + 1) * 512],
                                         wp[:, c * 128:(c + 1) * 128],
                                         XG[:, cs], start=True, stop=True)
                    vt = vp.tile([128, 1024], BF16, tag="vt")
                    if c2 == 0 or j == 0:
                        nc.vector.tensor_tensor(vt[:], pu[:],
                                                egtp[c2 * 2 + j][:], ALU.mult)
                    else:
                        cpt = vp.tile([128, 1024], BF16, tag="cpt")
                        nc.scalar.copy(cpt[:], pu[:])
                        nc.gpsimd.tensor_tensor(vt[:], cpt[:], egtpb[:],
                                                ALU.mult)
                    vts.append(vt)
                pus = pup.tile([128, 1024], F32, tag="pu")
                nc.tensor.matmul(pus[:, 0:512], wp[:, 512:640], XG[:, cs],
                                 start=True, stop=True)
                vs = vp.tile([128, 512], BF16, tag="vs")
                if c2 == 0:
                    nc.vector.tensor_tensor(vs[:], pus[:, 0:512], egt4[:],
                                            ALU.mult)
                else:
                    cps = vp.tile([128, 512], BF16, tag="cps")
                    nc.scalar.copy(cps[:], pus[:, 0:512])
                    nc.gpsimd.tensor_tensor(vs[:], cps[:], egt4b[:],
                                            ALU.mult)

                pg = plumb.tile([128, 512], F32, tag="pl")
                first = True
                for j in range(2):
                    for k in range(2):
                        nc.tensor.matmul(pg[:], wgoutd[:],
                                         vts[j][:, k * 512:(k + 1) * 512],
                                         start=first, stop=False,
                                         skip_group_check=True)
                        first = False
                nc.tensor.matmul(pg[:], wgoutd[:], vs[:],
                                 start=False, stop=False,
                                 skip_group_check=True)
                nc.tensor.matmul(pg[:], bp2[:], egu[:, cs],
                                 start=False, stop=True,
                                 skip_group_check=True)
                nc.scalar.activation(Gp[:, cs], pg[:], AF.Tanh,
                                     bias=bgout[:], scale=1.0)

            # kz = sum_i G_i * m_i, per chunk so chunk0's reduction runs
            # while chunk1's V/acc phase is still in flight. nlo/nh0 are
            # base-0 tiles so the adds are walrus-legal.
            nlo = wk.tile([64, 1024], BF16, tag="nlo")
            nh0 = wk.tile([64, 1024], BF16, tag="nh0")
            for c2 in range(2):
                cs = slice(c2 * 512, (c2 + 1) * 512)
                eng_a = nc.gpsimd if c2 == 0 else nc.vector
                eng_b = nc.vector if c2 == 0 else nc.gpsimd
                eng_a.tensor_tensor(nlo[:, cs], Gp[0:64, cs], m[0:64, cs],
                                    ALU.mult)
                eng_b.tensor_tensor(nh0[:, cs], Gp[64:128, cs],
                                    m[64:128, cs], ALU.mult)
                eng_a.tensor_tensor(kz[c2 * 64:(c2 + 1) * 64, :],
                                    nlo[:, cs], nh0[:, cs], ALU.add)
            return kh, kz

        third = 1.0 / 3.0

        for t in range(n_steps):
            bf = [(wfin, hb[:], False)]
            bg = [(wgin, zb[:], False)]
            k1h, k1z = vf(bf, bg, _dx_stage_index(t, 0), "1")
            k2h, k2z = vf(bf + [(wfin3, k1h[:], False)],
                          bg + [(wgin3, k1z[:], False)],
                          _dx_stage_index(t, 1), "2")
            k3h, k3z = vf(bf + [(wfinm3, k1h[:], False), (wfin, k2h[:], False)],
                          bg + [(wginm3, k1z[:], False), (wgin, k2z[:], False)],
                          _dx_stage_index(t, 2), "3")
            k4h, k4z = vf(bf + [(wfin, k1h[:], False), (wfinm1, k2h[:], False),
                                (wfin, k3h[:], False)],
                          bg + [(wgin, k1z[:], False), (wginm1, k2z[:], False),
                                (wgin, k3z[:], False)],
                          _dx_stage_index(t, 3), "4")

            # y' = y + (k1 + 3(k2+k3) + k4) / 8
            v1h = wk.tile([128, 512], BF16, tag="v1h")
            v1z = wk.tile([128, 512], BF16, tag="v1z")
            v2h = wk.tile([128, 512], BF16, tag="v2h")
            v2z = wk.tile([128, 512], BF16, tag="v2z")
            v3h = wk.tile([128, 512], BF16, tag="v3h")
            v3z = wk.tile([128, 512], BF16, tag="v3z")
            nc.gpsimd.tensor_tensor(v1h[:], k2h[:], k3h[:], ALU.add)
            nc.vector.scalar_tensor_tensor(v2h[:], v1h[:], 3.0, k1h[:],
                                           ALU.mult, ALU.add)
            nc.gpsimd.tensor_tensor(v1z[:], k2z[:], k3z[:], ALU.add)
            nc.vector.scalar_tensor_tensor(v2z[:], v1z[:], 3.0, k1z[:],
                                           ALU.mult, ALU.add)
            Hn = st.tile([128, 512], F32, tag="Hm")
            Zn = st.tile([128, 512], F32, tag="Zm")
            hbn = st.tile([128, 512], BF16, tag="hb")
            zbn = st.tile([128, 512], BF16, tag="zb")
            nc.vector.tensor_tensor(v3z[:], v2z[:], k4z[:], ALU.add)
            nc.vector.scalar_tensor_tensor(Zn[:], v3z[:], 0.125, Zm[:],
                                           ALU.mult, ALU.add)
            nc.vector.tensor_copy(zbn[:], Zn[:])
            nc.gpsimd.tensor_tensor(v3h[:], v2h[:], k4h[:], ALU.add)
            nc.vector.scalar_tensor_tensor(Hn[:], v3h[:], 0.125, Hm[:],
                                           ALU.mult, ALU.add)
            nc.gpsimd.tensor_copy(hbn[:], Hn[:])
            nc.sync.dma_start(ZOUT[t], Zn[:])
            Hm, Zm, hb, zb = Hn, Zn, hbn, zbn

    nc.compile()
    return nc


def _fold(a):
    """[64, 1024] -> folded [128, 512]."""
    return np.concatenate([a[:, 0:512], a[:, 512:1024]], axis=0)


def _prep_shared(inputs):
    f32 = np.float32
    bf16 = np.dtype("bfloat16") if hasattr(np, "bfloat16") else None
    import ml_dtypes
    bf16 = ml_dtypes.bfloat16

    Eg = np.asarray(inputs["Eg"], f32)
    W_pool = np.asarray(inputs["W_pool"], f32)
    b_pool = np.asarray(inputs["b_pool"], f32)

    logits = Eg @ Eg.T
    r = np.maximum(logits, 0.0)
    e = np.exp(r - r.max(axis=1, keepdims=True))
    A = (e / e.sum(axis=1, keepdims=True)).astype(f32)
    AT = np.ascontiguousarray(A.T)

    WP = np.ascontiguousarray(
        np.transpose(W_pool, (1, 2, 0, 3)).reshape(KCH * HID, EMB * HID)
    ).astype(f32)

    n_of_tok = np.tile(np.arange(N), BLOC)

    EGU = np.ascontiguousarray(Eg.T[:, n_of_tok]).astype(f32)  # [10, 1024]
    # mask pair tiles: EGTP[c2*2+j][dd*64+r, k*512+t] = Eg[n(c2*512+t), 2*(2j+k)+dd]
    EGTP4 = np.empty((2, 2, 128, 1024), f32)   # [c2][j]
    EGT4 = np.empty((2, 128, 512), f32)
    for c2 in range(2):
        nt = n_of_tok[c2 * 512:(c2 + 1) * 512]
        for j in range(2):
            for k in range(2):
                c = 2 * j + k
                for dd in range(2):
                    EGTP4[c2, j, dd * 64:(dd + 1) * 64,
                          k * 512:(k + 1) * 512] = Eg[nt, 2 * c + dd][None, :]
        for dd in range(2):
            EGT4[c2, dd * 64:(dd + 1) * 64, :] = Eg[nt, 8 + dd][None, :]

    perm = np.empty(HID * CIN, np.int64)
    for i in range(CIN):
        for hh in range(HID):
            perm[i * HID + hh] = hh * CIN + i

    def bd(w):
        out = np.zeros((128, 128), f32)
        out[0:64, 0:64] = w
        out[64:128, 64:128] = w
        return out

    def halfpad(w, top):
        out = np.zeros((128, 128), f32)
        if top:
            out[0:64, :] = w
        else:
            out[64:128, :] = w
        return out

    Wf_out_p = np.asarray(inputs["Wf_out"], f32)[:, perm]
    bf_out_p = np.asarray(inputs["bf_out"], f32)[perm]
    Wg_out_p = np.asarray(inputs["Wg_out"], f32)[:, perm]
    bg_out_p = np.asarray(inputs["bg_out"], f32)[perm]

    def b16(x):
        return np.ascontiguousarray(x).astype(bf16)

    wfin_bd = bd(np.asarray(inputs["Wf_in"], f32))
    wgin_bd = bd(np.asarray(inputs["Wg_in"], f32))
    shared = {
        "WFIN": b16(wfin_bd),
        "WFIN3": b16(wfin_bd / 3.0),
        "WFINM3": b16(-wfin_bd / 3.0),
        "WFINM1": b16(-wfin_bd),
        "WFHID": b16(bd(np.asarray(inputs["Wf_hid"], f32))),
        "WGIN": b16(wgin_bd),
        "WGIN3": b16(wgin_bd / 3.0),
        "WGINM3": b16(-wgin_bd / 3.0),
        "WGINM1": b16(-wgin_bd),
        "WFOUT_A": b16(halfpad(Wf_out_p, True)),
        "WFOUT_B": b16(halfpad(Wf_out_p, False)),
        "WGOUTD": b16(np.concatenate([Wg_out_p, Wg_out_p], axis=0)),
        "BP2": b16(b_pool @ Wg_out_p),
        "AT0": b16(AT[0:128, :]),
        "AT1": b16(AT[128:256, :]),
        "WP": b16(WP),
        "EGU": b16(EGU),
        "EGTP": np.ascontiguousarray(
            np.stack([EGTP4[0, 0], EGTP4[0, 1], EGTP4[1, 0]])),
        "EGTPB": b16(EGTP4[1, 1][None, :, :]),
        "EGT4": np.ascontiguousarray(EGT4[0][None, :, :]),
        "EGT4B": b16(EGT4[1][None, :, :]),
        "IDENT": b16(np.eye(64, dtype=f32)),
        "BFIN2": np.tile(np.asarray(inputs["bf_in"], f32), 2)[:, None],
        "BFHID2": np.tile(np.asarray(inputs["bf_hid"], f32), 2)[:, None],
        "BGIN2": np.tile(np.asarray(inputs["bg_in"], f32), 2)[:, None],
        "BFOUT": bf_out_p[:, None].astype(f32),
        "BGOUT": bg_out_p[:, None].astype(f32),
    }
    return shared


def _prep_core(inputs, core, n_steps=NSTEP):
    f32 = np.float32
    import ml_dtypes
    bf16 = ml_dtypes.bfloat16

    ca = np.asarray(inputs["coeff_a"], f32)
    cb = np.asarray(inputs["coeff_b"], f32)
    cc = np.asarray(inputs["coeff_two_c"], f32)
    cd = np.asarray(inputs["coeff_three_d"], f32)
    W_h = np.asarray(inputs["W_h"], f32)
    b_h = np.asarray(inputs["b_h"], f32)
    W_z = np.asarray(inputs["W_z"], f32)
    b_z = np.asarray(inputs["b_z"], f32)

    bsl = slice(core * BLOC, (core + 1) * BLOC)
    x0 = ca[bsl, :, 0, :]                       # [4, 256, 2]
    h0 = (x0 @ W_h + b_h).reshape(TOK, HID).T   # [64, 1024]
    z0 = (x0 @ W_z + b_z).reshape(TOK, HID).T

    HZF = np.concatenate([_fold(h0), _fold(z0)], axis=1)  # [128, 1024]

    DXB = np.empty((NSTAGE, 128, TOK), f32)
    maxidx = T - 2
    for si in range(NSTAGE):
        tt, s = si // 3, si % 3
        tval = tt + s / 3.0
        idx = min(int(np.floor(tval + 1e-9)), maxidx)
        frac = f32(tval - idx)
        dx = cb[bsl, :, idx, :] + (cc[bsl, :, idx, :]
                                   + cd[bsl, :, idx, :] * frac) * frac
        dx = dx.reshape(TOK, CIN)
        DXB[si, 0:64, :] = dx[:, 0][None, :]
        DXB[si, 64:128, :] = dx[:, 1][None, :]

    return {
        "HZF": HZF,
        "HZB": HZF.astype(bf16),
        "DXB": DXB.astype(bf16),
    }, (x0 @ W_z + b_z)


def kernel(**inputs):
    global LAST_RESULT
    from concourse.bass_utils import run_bass_kernel_spmd

    n_steps = int(os.environ.get("GCDE_NSTEPS", NSTEP))
    key = n_steps
    if key not in _KERNEL_CACHE:
        _KERNEL_CACHE[key] = _build(n_steps)
    nc = _KERNEL_CACHE[key]

    shared = _prep_shared(inputs)
    in_maps = []
    z0_full = np.empty((B, N, HID), np.float32)
    for core in range(NCORES):
        per, z0c = _prep_core(inputs, core, n_steps)
        z0_full[core * BLOC:(core + 1) * BLOC] = z0c
        in_maps.append({**shared, **per})

    kw = {}
    if os.environ.get("GCDE_TRACE"):
        kw = dict(trace=True, tmpdir=os.environ.get("GCDE_TRACE_DIR") or None)
    res = run_bass_kernel_spmd(nc, in_maps, list(range(NCORES)), **kw)
    LAST_RESULT = res

    out = np.empty((B, N, T, HID), np.float32)
    out[:, :, 0, :] = z0_full
    for core in range(NCORES):
        Z = res.results[core]["ZOUT"][:n_steps]  # [n_steps, 128, 512]
        zt = np.concatenate([Z[:, 0:64, :], Z[:, 64:128, :]], axis=2)
        zt = zt.transpose(0, 2, 1).reshape(n_steps, BLOC, N, HID)
        for t in range(n_steps):
            out[core * BLOC:(core + 1) * BLOC, :, t + 1, :] = zt[t]
        if n_steps < NSTEP:
            out[:, :, n_steps + 1:, :] = 0.0
    return out


# revision 21
# speedup vs baseline: 1.1987x; 1.1987x over previous
"""NeuralGCDE Trainium2 kernel, v2.

Data-parallel over batch B=32 across 8 NeuronCores (B_loc=4/core, weights
replicated, no inter-core traffic). Per core the RK4 scan (12 steps x 4
stages) runs on-device.

Layout/dtype strategy (vs the fp32 v1):
  - bf16 everywhere except: PSUM (always fp32), the fp32 master state
    Hm/Zm, per-partition biases, and the DVE-path Eg masks (fp32 to match
    fp32 PSUM operands).
  - Full-width elementwise: engine op cost scales with free-dim only, so
    every op uses all 128 partitions (folded layouts).
  - u2/u3/u4 stage-state combines eliminated via linearity of the input
    layers: each stage's Wf_in/Wg_in PSUM accumulates a seed matmul on the
    bf16 state shadow plus pre-scaled-weight matmuls on the k tiles.
  - F/G/m in i-major rows [128=(i,h)], tokens 0:1024 in columns; kh/kz
    chunk-folded [128=(chunk,h), 512] via equal-base-partition adds
    (walrus requires equal SBUF operand bases), per-chunk for kz so
    chunk0's reduction overlaps chunk1's V/acc phase.
  - U outputs pairwise in 2-bank PSUM tiles [128, 1024]; a single DVE op
    per pair fuses the PSUM->SBUF move with the Eg mask multiply; two of
    the six mask slots run ACT copy-cast -> GpSimd bf16 multiply instead
    to spread load.
"""
import sys
import os
import numpy as np

if "/opt/trn_rl_repo" not in sys.path:
    sys.path.insert(0, "/opt/trn_rl_repo")

B, N, T, CIN, HID, EMB, KCH = 32, 256, 13, 2, 64, 10, 2
NCORES = 8
BLOC = B // NCORES          # 4
TOK = BLOC * N              # 1024
NSTEP = T - 1               # 12
NSTAGE = 3 * NSTEP + 1      # 37 distinct spline-derivative tensors

_KERNEL_CACHE = {}
LAST_RESULT = None


def _dx_stage_index(t, s):
    if s < 3:
        return 3 * t + s
    return 3 * (t + 1) if (t + 1) < NSTEP else 3 * NSTEP


def _build(n_steps=NSTEP):
    import concourse.bacc as bacc
    import concourse.tile as tile
    from concourse import mybir
    from contextlib import ExitStack

    F32 = mybir.dt.float32
    BF16 = mybir.dt.bfloat16
    AF = mybir.ActivationFunctionType
    ALU = mybir.AluOpType

    nc = bacc.Bacc("TRN2", target_bir_lowering=False, debug=False,
                   num_devices=NCORES)

    def din(name, shape, dt=BF16):
        return nc.dram_tensor(name, shape, dt, kind="ExternalInput").ap()

    WFIN = din("WFIN", [128, 128])
    WFIN3 = din("WFIN3", [128, 128])
    WFINM3 = din("WFINM3", [128, 128])
    WFINM1 = din("WFINM1", [128, 128])
    WFHID = din("WFHID", [128, 128])
    WGIN = din("WGIN", [128, 128])
    WGIN3 = din("WGIN3", [128, 128])
    WGINM3 = din("WGINM3", [128, 128])
    WGINM1 = din("WGINM1", [128, 128])
    WFOUT_A = din("WFOUT_A", [128, 128])
    WFOUT_B = din("WFOUT_B", [128, 128])
    WGOUTD = din("WGOUTD", [128, 128])
    BP2 = din("BP2", [10, 128])
    AT0 = din("AT0", [128, 256])
    AT1 = din("AT1", [128, 256])
    WP = din("WP", [128, 640])
    EGU = din("EGU", [10, 1024])
    EGTP = din("EGTP", [3, 128, 1024], F32)   # DVE-path pair masks
    EGTPB = din("EGTPB", [1, 128, 1024])      # pool-path pair mask (c2=1,j=1)
    EGT4 = din("EGT4", [1, 128, 512], F32)    # c2=0 single mask (DVE)
    EGT4B = din("EGT4B", [1, 128, 512])       # c2=1 single mask (pool)
    IDENT = din("IDENT", [64, 64])
    BFIN2 = din("BFIN2", [128, 1], F32)
    BFHID2 = din("BFHID2", [128, 1], F32)
    BGIN2 = din("BGIN2", [128, 1], F32)
    BFOUT = din("BFOUT", [128, 1], F32)
    BGOUT = din("BGOUT", [128, 1], F32)
    HZF = din("HZF", [128, 1024], F32)
    HZB = din("HZB", [128, 1024])
    DXB = din("DXB", [NSTAGE, 128, 1024])
    ZOUT = nc.dram_tensor("ZOUT", [NSTEP, 128, 512], F32,
                          kind="ExternalOutput").ap()

    _ts = bool(os.environ.get("GCDE_TRACESIM"))
    with tile.TileContext(nc, trace_sim=_ts) as tc, ExitStack() as ctx:
        cp = ctx.enter_context(tc.tile_pool(name="const", bufs=1))
        wk = ctx.enter_context(tc.tile_pool(name="work", bufs=3))
        mk = ctx.enter_context(tc.tile_pool(name="mk", bufs=2))
        st = ctx.enter_context(tc.tile_pool(name="state", bufs=2))
        vp = ctx.enter_context(tc.tile_pool(name="vpool", bufs=4))
        dxp = ctx.enter_context(tc.tile_pool(name="dxp", bufs=3))
        pfp = ctx.enter_context(tc.tile_pool(name="pfp", bufs=2, space="PSUM"))
        plumb = ctx.enter_context(tc.tile_pool(name="plumb", bufs=2,
                                               space="PSUM"))
        pup = ctx.enter_context(tc.tile_pool(name="pup", bufs=2, space="PSUM"))

        def cload(src, shape, tag, dt=BF16):
            t = cp.tile(shape, dt, tag=tag)
            nc.sync.dma_start(t[:], src)
            return t

        wfin = cload(WFIN, [128, 128], "wfin")
        wfin3 = cload(WFIN3, [128, 128], "wfin3")
        wfinm3 = cload(WFINM3, [128, 128], "wfinm3")
        wfinm1 = cload(WFINM1, [128, 128], "wfinm1")
        wfhid = cload(WFHID, [128, 128], "wfhid")
        wgin = cload(WGIN, [128, 128], "wgin")
        wgin3 = cload(WGIN3, [128, 128], "wgin3")
        wginm3 = cload(WGINM3, [128, 128], "wginm3")
        wginm1 = cload(WGINM1, [128, 128], "wginm1")
        wfout_a = cload(WFOUT_A, [128, 128], "wfout_a")
        wfout_b = cload(WFOUT_B, [128, 128], "wfout_b")
        wgoutd = cload(WGOUTD, [128, 128], "wgoutd")
        bp2 = cload(BP2, [10, 128], "bp2")
        at0 = cload(AT0, [128, 256], "at0")
        at1 = cload(AT1, [128, 256], "at1")
        wp = cload(WP, [128, 640], "wp")
        egu = cload(EGU, [10, 1024], "egu")
        ident = cload(IDENT, [64, 64], "ident")
        bfin2 = cload(BFIN2, [128, 1], "bfin2", F32)
        bfhid2 = cload(BFHID2, [128, 1], "bfhid2", F32)
        bgin2 = cload(BGIN2, [128, 1], "bgin2", F32)
        bfout = cload(BFOUT, [128, 1], "bfout", F32)
        bgout = cload(BGOUT, [128, 1], "bgout", F32)
        egtp = []
        for i in range(3):
            t = cp.tile([128, 1024], F32, tag=f"egtp{i}")
            nc.sync.dma_start(t[:], EGTP[i])
            egtp.append(t)
        egtpb = cp.tile([128, 1024], BF16, tag="egtpb")
        nc.sync.dma_start(egtpb[:], EGTPB[0])
        egt4 = cp.tile([128, 512], F32, tag="egt4")
        nc.sync.dma_start(egt4[:], EGT4[0])
        egt4b = cp.tile([128, 512], BF16, tag="egt4b")
        nc.sync.dma_start(egt4b[:], EGT4B[0])

        Hm = st.tile([128, 512], F32, tag="Hm")
        Zm = st.tile([128, 512], F32, tag="Zm")
        hb = st.tile([128, 512], BF16, tag="hb")
        zb = st.tile([128, 512], BF16, tag="zb")
        nc.sync.dma_start(Hm[:], HZF[:, 0:512])
        nc.sync.dma_start(Zm[:], HZF[:, 512:1024])
        nc.sync.dma_start(hb[:], HZB[:, 0:512])
        nc.sync.dma_start(zb[:], HZB[:, 512:1024])

        def _acc_terms(out, terms):
            """Accumulate (weight, rhs, halved) into a PSUM stage input.
            halved k-terms contract per 64-row chunk so each kz/kh half
            unblocks the next stage independently."""
            subs = []
            for w, rhs, halved in terms:
                if halved:
                    subs.append((w[0:64, :], rhs[0:64, :]))
                    subs.append((w[64:128, :], rhs[64:128, :]))
                else:
                    subs.append((w[:], rhs))
            for i, (wap, rap) in enumerate(subs):
                nc.tensor.matmul(out[:], wap, rap, start=(i == 0),
                                 stop=(i == len(subs) - 1),
                                 skip_group_check=True)

        def vf(fterms, gterms, sidx, ktag):
            """fterms/gterms: [(weight_tile, rhs_AP)] accumulated into the
            stage input PSUMs (linearity of the input layers replaces the
            explicit u2/u3/u4 state combines). Returns (kh, kz) [128,512]
            bf16 chunk-folded; kh completes early (f path), kz late."""
            dxb = dxp.tile([128, 1024], BF16, tag="dxb")
            nc.sync.dma_start(dxb[:], DXB[sidx])

            kh = mk.tile([128, 512], BF16, tag="kh" + ktag)
            kz = mk.tile([128, 512], BF16, tag="kz" + ktag)

            # ---- f path ----
            pf1 = pfp.tile([128, 512], F32, tag="pf")
            _acc_terms(pf1, fterms)
            x1 = wk.tile([128, 512], BF16, tag="x1")
            nc.scalar.activation(x1[:], pf1[:], AF.Relu, bias=bfin2[:],
                                 scale=1.0)

            pf2 = pfp.tile([128, 512], F32, tag="pf")
            nc.tensor.matmul(pf2[:], wfhid[:], x1[:], start=True, stop=True)
            x2 = wk.tile([128, 512], BF16, tag="x2")
            nc.scalar.activation(x2[:], pf2[:], AF.Relu, bias=bfhid2[:],
                                 scale=1.0)

            # F in i-major rows, chunk-in-columns [128, 1024]
            Fp = wk.tile([128, 1024], BF16, tag="Fp")
            for half, wo in ((0, wfout_a), (1, wfout_b)):
                pF = pfp.tile([128, 512], F32, tag="pf")
                nc.tensor.matmul(pF[:], wo[:], x2[:], start=True, stop=True)
                nc.scalar.activation(Fp[:, half * 512:(half + 1) * 512],
                                     pF[:], AF.Tanh, bias=bfout[:], scale=1.0)

            # m = F * dX as two equal-base half ops; mh2 = base-0 shadow of
            # the hi half for the i-reduction (walrus: SB+SB operand bases
            # must match).
            m = wk.tile([128, 1024], BF16, tag="m")
            nc.gpsimd.tensor_tensor(m[0:64, :], Fp[0:64, :], dxb[0:64, :],
                                    ALU.mult)
            nc.gpsimd.tensor_tensor(m[64:128, :], Fp[64:128, :],
                                    dxb[64:128, :], ALU.mult)
            mh2 = wk.tile([64, 1024], BF16, tag="mh2")
            nc.gpsimd.tensor_copy(mh2[:], m[64:128, :])
            for c2 in range(2):
                cs = slice(c2 * 512, (c2 + 1) * 512)
                nc.gpsimd.tensor_tensor(kh[c2 * 64:(c2 + 1) * 64, :],
                                        m[0:64, cs], mh2[:, cs], ALU.add)

            # ---- g path ----
            pq = plumb.tile([128, 512], F32, tag="pl")
            _acc_terms(pq, gterms)
            XG = wk.tile([128, 1024], BF16, tag="XG")
            nc.scalar.activation(XG[0:64, 0:512], pq[0:64, :], AF.Relu,
                                 bias=bgin2[0:64], scale=1.0)
            nc.vector.tensor_scalar(XG[0:64, 512:1024], pq[64:128, :],
                                    bgin2[64:128], 0.0, ALU.add, ALU.max)

            # transposes: x [64(i), tok] -> xtt [128(m), (b,mh,i)]
            ptt = plumb.tile([128, 512], BF16, tag="pl")
            for b in range(BLOC):
                for mh in range(2):
                    nc.tensor.transpose(
                        ptt[:, (b * 2 + mh) * 64:(b * 2 + mh + 1) * 64],
                        XG[0:64, b * 256 + mh * 128: b * 256 + (mh + 1) * 128],
                        ident[:],
                    )
            xtt = wk.tile([128, 512], BF16, tag="xtt")
            nc.vector.tensor_copy(xtt[:, 0:256], ptt[:, 0:256])
            nc.scalar.copy(xtt[:, 256:512], ptt[:, 256:512])

            # support matmul: Ax[i, (b,n)] accumulated over m-halves
            for pi in range(2):
                px = plumb.tile([64, 512], F32, tag="pl")
                for bb in range(2):
                    b = pi * 2 + bb
                    bs = slice(bb * 256, (bb + 1) * 256)
                    nc.tensor.matmul(px[:, bs],
                                     xtt[:, (b * 2) * 64:(b * 2 + 1) * 64],
                                     at0[:], start=True, stop=False)
                    nc.tensor.matmul(px[:, bs],
                                     xtt[:, (b * 2 + 1) * 64:(b * 2 + 2) * 64],
                                     at1[:], start=False, stop=True)
                dst = slice(pi * 512, (pi + 1) * 512)
                if pi == 0:
                    nc.scalar.copy(XG[64:128, dst], px[:])
                else:
                    nc.vector.tensor_copy(XG[64:128, dst], px[:])

            # adaptive conv: U pairs -> masked V -> accumulating projection.
            # c2=0's masks run direct on DVE (PSUM read); c2=1's pair masks
            # go ACT copy-cast -> pool bf16 mult to spread the load.
            Gp = wk.tile([128, 1024], BF16, tag="Gp")
            for c2 in range(2):
                cs = slice(c2 * 512, (c2 + 1) * 512)
                vts = []
                for j in range(2):
                    pu = pup.tile([128, 1024], F32, tag="pu")
                    for k in range(2):
                        c = 2 * j + k
                        nc.tensor.matmul(pu[:, k * 512:(k + 1) * 512],
                                         wp[:, c * 128:(c + 1) * 128],
                                         XG[:, cs], start=True, stop=True)
                    vt = vp.tile([128, 1024], BF16, tag="vt")
                    if c2 == 1 or j == 0:
                        # DVE direct: (c2=0,j0)->0, (c2=1,j0)->1, (c2=1,j1)->2
                        nc.vector.tensor_tensor(vt[:], pu[:],
                                                egtp[0 if c2 == 0 else 1 + j][:],
                                                ALU.mult)
                    else:
                        cpt = vp.tile([128, 1024], BF16, tag="cpt")
                        nc.scalar.copy(cpt[:], pu[:])
                        nc.gpsimd.tensor_tensor(vt[:], cpt[:], egtpb[:],
                                                ALU.mult)
                    vts.append(vt)
                pus = pup.tile([128, 1024], F32, tag="pu")
                nc.tensor.matmul(pus[:, 0:512], wp[:, 512:640], XG[:, cs],
                                 start=True, stop=True)
                vs = vp.tile([128, 512], BF16, tag="vs")
                if c2 == 1:
                    nc.vector.tensor_tensor(vs[:], pus[:, 0:512], egt4[:],
                                            ALU.mult)
                else:
                    cps = vp.tile([128, 512], BF16, tag="cps")
                    nc.scalar.copy(cps[:], pus[:, 0:512])
                    nc.gpsimd.tensor_tensor(vs[:], cps[:], egt4b[:],
                                            ALU.mult)

                pg = plumb.tile([128, 512], F32, tag="pl")
                first = True
                for j in range(2):
                    for k in range(2):
                        nc.tensor.matmul(pg[:], wgoutd[:],
                                         vts[j][:, k * 512:(k + 1) * 512],
                                         start=first, stop=False,
                                         skip_group_check=True)
                        first = False
                nc.tensor.matmul(pg[:], wgoutd[:], vs[:],
                                 start=False, stop=False,
                                 skip_group_check=True)
                nc.tensor.matmul(pg[:], bp2[:], egu[:, cs],
                                 start=False, stop=True,
                                 skip_group_check=True)
                nc.scalar.activation(Gp[:, cs], pg[:], AF.Tanh,
                                     bias=bgout[:], scale=1.0)

            # kz = sum_i G_i * m_i, per chunk so chunk0's reduction runs
            # while chunk1's V/acc phase is still in flight. nlo/nh0 are
            # base-0 tiles so the adds are walrus-legal.
            nlo = wk.tile([64, 1024], BF16, tag="nlo")
            nh0 = wk.tile([64, 1024], BF16, tag="nh0")
            for c2 in range(2):
                cs = slice(c2 * 512, (c2 + 1) * 512)
                eng_a = nc.gpsimd if c2 == 0 else nc.vector
                eng_b = nc.vector if c2 == 0 else nc.gpsimd
                eng_a.tensor_tensor(nlo[:, cs], Gp[0:64, cs], m[0:64, cs],
                                    ALU.mult)
                eng_b.tensor_tensor(nh0[:, cs], Gp[64:128, cs],
                                    m[64:128, cs], ALU.mult)
                eng_a.tensor_tensor(kz[c2 * 64:(c2 + 1) * 64, :],
                                    nlo[:, cs], nh0[:, cs], ALU.add)
            return kh, kz

        third = 1.0 / 3.0

        for t in range(n_steps):
            bf = [(wfin, hb[:], False)]
            bg = [(wgin, zb[:], False)]
            k1h, k1z = vf(bf, bg, _dx_stage_index(t, 0), "1")
            k2h, k2z = vf(bf + [(wfin3, k1h[:], False)],
                          bg + [(wgin3, k1z[:], False)],
                          _dx_stage_index(t, 1), "2")
            k3h, k3z = vf(bf + [(wfinm3, k1h[:], False), (wfin, k2h[:], False)],
                          bg + [(wginm3, k1z[:], False), (wgin, k2z[:], False)],
                          _dx_stage_index(t, 2), "3")
            k4h, k4z = vf(bf + [(wfin, k1h[:], False), (wfinm1, k2h[:], False),
                                (wfin, k3h[:], False)],
                          bg + [(wgin, k1z[:], False), (wginm1, k2z[:], False),
                                (wgin, k3z[:], False)],
                          _dx_stage_index(t, 3), "4")

            # y' = y + (k1 + 3(k2+k3) + k4) / 8
            v1h = wk.tile([128, 512], BF16, tag="v1h")
            v1z = wk.tile([128, 512], BF16, tag="v1z")
            v2h = wk.tile([128, 512], BF16, tag="v2h")
            v2z = wk.tile([128, 512], BF16, tag="v2z")
            v3h = wk.tile([128, 512], BF16, tag="v3h")
            v3z = wk.tile([128, 512], BF16, tag="v3z")
            nc.gpsimd.tensor_tensor(v1h[:], k2h[:], k3h[:], ALU.add)
            nc.vector.scalar_tensor_tensor(v2h[:], v1h[:], 3.0, k1h[:],
                                           ALU.mult, ALU.add)
            nc.gpsimd.tensor_tensor(v1z[:], k2z[:], k3z[:], ALU.add)
            nc.vector.scalar_tensor_tensor(v2z[:], v1z[:], 3.0, k1z[:],
                                           ALU.mult, ALU.add)
            Hn = st.tile([128, 512], F32, tag="Hm")
            Zn = st.tile([128, 512], F32, tag="Zm")
            hbn = st.tile([128, 512], BF16, tag="hb")
            zbn = st.tile([128, 512], BF16, tag="zb")
            nc.vector.tensor_tensor(v3z[:], v2z[:], k4z[:], ALU.add)
            nc.vector.scalar_tensor_tensor(Zn[:], v3z[:], 0.125, Zm[:],
                                           ALU.mult, ALU.add)
            nc.vector.tensor_copy(zbn[:], Zn[:])
            nc.gpsimd.tensor_tensor(v3h[:], v2h[:], k4h[:], ALU.add)
            nc.vector.scalar_tensor_tensor(Hn[:], v3h[:], 0.125, Hm[:],
                                           ALU.mult, ALU.add)
            nc.gpsimd.tensor_copy(hbn[:], Hn[:])
            nc.sync.dma_start(ZOUT[t], Zn[:])
            Hm, Zm, hb, zb = Hn, Zn, hbn, zbn

    nc.compile()
    return nc


def _fold(a):
    """[64, 1024] -> folded [128, 512]."""
    return np.concatenate([a[:, 0:512], a[:, 512:1024]], axis=0)


def _prep_shared(inputs):
    f32 = np.float32
    bf16 = np.dtype("bfloat16") if hasattr(np, "bfloat16") else None
    import ml_dtypes
    bf16 = ml_dtypes.bfloat16

    Eg = np.asarray(inputs["Eg"], f32)
    W_pool = np.asarray(inputs["W_pool"], f32)
    b_pool = np.asarray(inputs["b_pool"], f32)

    logits = Eg @ Eg.T
    r = np.maximum(logits, 0.0)
    e = np.exp(r - r.max(axis=1, keepdims=True))
    A = (e / e.sum(axis=1, keepdims=True)).astype(f32)
    AT = np.ascontiguousarray(A.T)

    WP = np.ascontiguousarray(
        np.transpose(W_pool, (1, 2, 0, 3)).reshape(KCH * HID, EMB * HID)
    ).astype(f32)

    n_of_tok = np.tile(np.arange(N), BLOC)

    EGU = np.ascontiguousarray(Eg.T[:, n_of_tok]).astype(f32)  # [10, 1024]
    # mask pair tiles: EGTP[c2*2+j][dd*64+r, k*512+t] = Eg[n(c2*512+t), 2*(2j+k)+dd]
    EGTP4 = np.empty((2, 2, 128, 1024), f32)   # [c2][j]
    EGT4 = np.empty((2, 128, 512), f32)
    for c2 in range(2):
        nt = n_of_tok[c2 * 512:(c2 + 1) * 512]
        for j in range(2):
            for k in range(2):
                c = 2 * j + k
                for dd in range(2):
                    EGTP4[c2, j, dd * 64:(dd + 1) * 64,
                          k * 512:(k + 1) * 512] = Eg[nt, 2 * c + dd][None, :]
        for dd in range(2):
            EGT4[c2, dd * 64:(dd + 1) * 64, :] = Eg[nt, 8 + dd][None, :]

    perm = np.empty(HID * CIN, np.int64)
    for i in range(CIN):
        for hh in range(HID):
            perm[i * HID + hh] = hh * CIN + i

    def bd(w):
        out = np.zeros((128, 128), f32)
        out[0:64, 0:64] = w
        out[64:128, 64:128] = w
        return out

    def halfpad(w, top):
        out = np.zeros((128, 128), f32)
        if top:
            out[0:64, :] = w
        else:
            out[64:128, :] = w
        return out

    Wf_out_p = np.asarray(inputs["Wf_out"], f32)[:, perm]
    bf_out_p = np.asarray(inputs["bf_out"], f32)[perm]
    Wg_out_p = np.asarray(inputs["Wg_out"], f32)[:, perm]
    bg_out_p = np.asarray(inputs["bg_out"], f32)[perm]

    def b16(x):
        return np.ascontiguousarray(x).astype(bf16)

    wfin_bd = bd(np.asarray(inputs["Wf_in"], f32))
    wgin_bd = bd(np.asarray(inputs["Wg_in"], f32))
    shared = {
        "WFIN": b16(wfin_bd),
        "WFIN3": b16(wfin_bd / 3.0),
        "WFINM3": b16(-wfin_bd / 3.0),
        "WFINM1": b16(-wfin_bd),
        "WFHID": b16(bd(np.asarray(inputs["Wf_hid"], f32))),
        "WGIN": b16(wgin_bd),
        "WGIN3": b16(wgin_bd / 3.0),
        "WGINM3": b16(-wgin_bd / 3.0),
        "WGINM1": b16(-wgin_bd),
        "WFOUT_A": b16(halfpad(Wf_out_p, True)),
        "WFOUT_B": b16(halfpad(Wf_out_p, False)),
        "WGOUTD": b16(np.concatenate([Wg_out_p, Wg_out_p], axis=0)),
        "BP2": b16(b_pool @ Wg_out_p),
        "AT0": b16(AT[0:128, :]),
        "AT1": b16(AT[128:256, :]),
        "WP": b16(WP),
        "EGU": b16(EGU),
        "EGTP": np.ascontiguousarray(
            np.stack([EGTP4[0, 0], EGTP4[1, 0], EGTP4[1, 1]])),
        "EGTPB": b16(EGTP4[0, 1][None, :, :]),
        "EGT4": np.ascontiguousarray(EGT4[1][None, :, :]),
        "EGT4B": b16(EGT4[0][None, :, :]),
        "IDENT": b16(np.eye(64, dtype=f32)),
        "BFIN2": np.tile(np.asarray(inputs["bf_in"], f32), 2)[:, None],
        "BFHID2": np.tile(np.asarray(inputs["bf_hid"], f32), 2)[:, None],
        "BGIN2": np.tile(np.asarray(inputs["bg_in"], f32), 2)[:, None],
        "BFOUT": bf_out_p[:, None].astype(f32),
        "BGOUT": bg_out_p[:, None].astype(f32),
    }
    return shared


def _prep_core(inputs, core, n_steps=NSTEP):
    f32 = np.float32
    import ml_dtypes
    bf16 = ml_dtypes.bfloat16

    ca = np.asarray(inputs["coeff_a"], f32)
    cb = np.asarray(inputs["coeff_b"], f32)
    cc = np.asarray(inputs["coeff_two_c"], f32)
    cd = np.asarray(inputs["coeff_three_d"], f32)
    W_h = np.asarray(inputs["W_h"], f32)
    b_h = np.asarray(inputs["b_h"], f32)
    W_z = np.asarray(inputs["W_z"], f32)
    b_z = np.asarray(inputs["b_z"], f32)

    bsl = slice(core * BLOC, (core + 1) * BLOC)
    x0 = ca[bsl, :, 0, :]                       # [4, 256, 2]
    h0 = (x0 @ W_h + b_h).reshape(TOK, HID).T   # [64, 1024]
    z0 = (x0 @ W_z + b_z).reshape(TOK, HID).T

    HZF = np.concatenate([_fold(h0), _fold(z0)], axis=1)  # [128, 1024]

    DXB = np.empty((NSTAGE, 128, TOK), f32)
    maxidx = T - 2
    for si in range(NSTAGE):
        tt, s = si // 3, si % 3
        tval = tt + s / 3.0
        idx = min(int(np.floor(tval + 1e-9)), maxidx)
        frac = f32(tval - idx)
        dx = cb[bsl, :, idx, :] + (cc[bsl, :, idx, :]
                                   + cd[bsl, :, idx, :] * frac) * frac
        dx = dx.reshape(TOK, CIN)
        DXB[si, 0:64, :] = dx[:, 0][None, :]
        DXB[si, 64:128, :] = dx[:, 1][None, :]

    return {
        "HZF": HZF,
        "HZB": HZF.astype(bf16),
        "DXB": DXB.astype(bf16),
    }, (x0 @ W_z + b_z)


def kernel(**inputs):
    global LAST_RESULT
    from concourse.bass_utils import run_bass_kernel_spmd

    n_steps = int(os.environ.get("GCDE_NSTEPS", NSTEP))
    key = n_steps
    if key not in _KERNEL_CACHE:
        _KERNEL_CACHE[key] = _build(n_steps)
    nc = _KERNEL_CACHE[key]

    shared = _prep_shared(inputs)
    in_maps = []
    z0_full = np.empty((B, N, HID), np.float32)
    for core in range(NCORES):
        per, z0c = _prep_core(inputs, core, n_steps)
        z0_full[core * BLOC:(core + 1) * BLOC] = z0c
        in_maps.append({**shared, **per})

    kw = {}
    if os.environ.get("GCDE_TRACE"):
        kw = dict(trace=True, tmpdir=os.environ.get("GCDE_TRACE_DIR") or None)
    res = run_bass_kernel_spmd(nc, in_maps, list(range(NCORES)), **kw)
    LAST_RESULT = res

    out = np.empty((B, N, T, HID), np.float32)
    out[:, :, 0, :] = z0_full
    for core in range(NCORES):
        Z = res.results[core]["ZOUT"][:n_steps]  # [n_steps, 128, 512]
        zt = np.concatenate([Z[:, 0:64, :], Z[:, 64:128, :]], axis=2)
        zt = zt.transpose(0, 2, 1).reshape(n_steps, BLOC, N, HID)
        for t in range(n_steps):
            out[core * BLOC:(core + 1) * BLOC, :, t + 1, :] = zt[t]
        if n_steps < NSTEP:
            out[:, :, n_steps + 1:, :] = 0.0
    return out
